# revision 27
# baseline (speedup 1.0000x reference)
"""MoE routing kernel for Trainium2, 8-core data-parallel.

Problem: nn_MORTM (moe_routing). Full inputs in, full output out.
Sharding: pure data-parallel over tokens (8192 tokens -> 8 cores x 1024).
Each core computes gate softmax + top-2 combine, all 8 routed experts
(dense, weighted by the combine matrix), and the shared expert for its
token slice. No collectives needed; output is a concat of slices.

Matmuls run as float32r (full PE rate at moving dim >= 256) except the
gate matmul, which stays fp32 so top-2 selection matches the fp32
reference ordering.
"""

import numpy as np

import concourse.bacc as bacc
import concourse.bass as bass
import concourse.masks as masks
import concourse.mybir as mybir
import concourse.tile as tile
from concourse.bass_utils import run_bass_kernel_spmd

F32 = mybir.dt.float32
F32R = mybir.dt.float32r
AF = mybir.ActivationFunctionType
ALU = mybir.AluOpType
AX = mybir.AxisListType

N_CORES = 8
USE_SILU = True   # sim_check flips this: CoreSim lacks the Silu LUT
ZERO_BIASES = False  # set by kernel() when every bias input is zero
T = 1024          # tokens per core
D = 1024          # d_model
INTER = 1024      # expert hidden
E = 8             # experts
TB = T // 128     # 128-token blocks
NT = T // 512     # 512-token tiles
DC = D // 128     # d chunks
IC = INTER // 128 # inter chunks
DT = D // 512     # 512-wide d tiles


def r32(ap):
    return ap.bitcast(F32R)


def emit(nc, tc, tensors):
    x_d = tensors["x"]
    gate_d = tensors["gate_w"]
    out_d = tensors["out"]

    xin = x_d.ap().rearrange("(tb p) d -> p tb d", p=128)
    outv = out_d.ap().rearrange("(tb p) d -> p tb d", p=128)

    ctx = tc.nc._emit_ctx  # ExitStack owned by build_nc
    singles = ctx.enter_context(tc.tile_pool(name="singles", bufs=1))
    psum = ctx.enter_context(tc.tile_pool(name="psum", bufs=8, space="PSUM"))
    tmp = ctx.enter_context(tc.tile_pool(name="tmp", bufs=2))
    big = ctx.enter_context(tc.tile_pool(name="big", bufs=1))
    wpool = ctx.enter_context(tc.tile_pool(name="wpool", bufs=24))
    hpool = ctx.enter_context(tc.tile_pool(name="hpool", bufs=1))
    iop = ctx.enter_context(tc.tile_pool(name="iop", bufs=6))

    ident = singles.tile([128, 128], F32)
    masks.make_identity(nc, ident[:])
    onesf = singles.tile([1, 128], F32)
    nc.vector.memset(onesf[:], 1.0)
    ones1 = singles.tile([1, 128], F32R)
    nc.vector.tensor_copy(ones1[:], onesf[:])

    # gate weights transposed: gwT[p, dc, e] = gate_w[e, dc*128+p]
    gwT = singles.tile([128, DC, E], F32)
    for dc in range(DC):
        nc.sync.dma_start(
            gwT[:, dc, :],
            gate_d.ap()[:, dc * 128:(dc + 1) * 128].rearrange("e p -> p e"),
        )

    # routed biases: b1s[p, e, ic] = b1[e, ic*128+p]
    b1s = b3s = sb1s = sb3s = b2r = sb2r = None
    if ZERO_BIASES:
        pass
    else:
        _load_biases = True
    b1s = singles.tile([128, E, IC], F32) if not ZERO_BIASES else None
    b3s = singles.tile([128, E, IC], F32) if not ZERO_BIASES else None
    for e in range(E if not ZERO_BIASES else 0):
        nc.sync.dma_start(
            b1s[:, e, :],
            tensors["b1"].ap()[e].rearrange("(ic p) -> p ic", p=128),
        )
        nc.sync.dma_start(
            b3s[:, e, :],
            tensors["b3"].ap()[e].rearrange("(ic p) -> p ic", p=128),
        )
    if not ZERO_BIASES:
        sb1s = singles.tile([128, IC], F32)
        nc.sync.dma_start(
            sb1s[:], tensors["sb1"].ap().rearrange("(ic p) -> p ic", p=128)
        )
        sb3s = singles.tile([128, IC], F32)
        nc.sync.dma_start(
            sb3s[:], tensors["sb3"].ap().rearrange("(ic p) -> p ic", p=128)
        )
    # row biases for the second matmul (added via K=1 matmul broadcast);
    # expert j's row lives on partition j.
    if not ZERO_BIASES:
        b2r = singles.tile([E, D], F32R)
        nc.sync.dma_start(b2r[:], tensors["b2"].ap().bitcast(F32R))
        sb2r = singles.tile([1, D], F32R)
        nc.sync.dma_start(
            sb2r[:],
            tensors["sb2"].ap().rearrange("(o d) -> o d", o=1).bitcast(F32R),
        )

    xt = big.tile([128, DC, T], F32R)     # xt[p, dc, t] = x[t, dc*128+p]
    comb = big.tile([128, TB, E], F32)   # combine matrix
    comb_t = (
        None if ZERO_BIASES else big.tile([8, T], F32R)
    )  # combine transposed [expert, token]

    # ---- per token block: load x, PE-transpose (fp32 stage + f32r copy),
    #      gate scores from the fp32 stage -> softmax -> top2 -> combine ----
    xpool_cm = tc.tile_pool(name="xnat", bufs=2)
    xpool = xpool_cm.__enter__()
    for tb in range(TB):
        xnat = xpool.tile([128, D], F32, tag="xnat")
        nc.sync.dma_start(xnat[:], xin[:, tb, :])
        xstage = xpool.tile([128, DC, 128], F32, tag="xstage")
        for dc in range(DC):
            pt = psum.tile([128, 512], F32, tag="ps")
            nc.tensor.transpose(
                pt[:, :128], xnat[:, dc * 128:(dc + 1) * 128], ident[:]
            )
            nc.vector.tensor_copy(xstage[:, dc, :], pt[:, :128])
            nc.vector.tensor_copy(xt[:, dc, tb * 128:(tb + 1) * 128], xstage[:, dc, :])
        ps = psum.tile([128, 512], F32, tag="ps")
        for dc in range(DC):
            nc.tensor.matmul(
                ps[:, :E],
                xstage[:, dc, :],
                gwT[:, dc, :],
                start=(dc == 0),
                stop=(dc == DC - 1),
            )
        nmx = tmp.tile([128, 1], F32, tag="nmx")
        nc.vector.tensor_reduce(nmx[:], ps[:, :E], axis=AX.X, op=ALU.max, negate=True)
        ex = tmp.tile([128, E], F32, tag="ex")
        nc.scalar.activation(ex[:], ps[:, :E], AF.Exp, bias=nmx[:])
        ssum = tmp.tile([128, 1], F32, tag="ssum")
        nc.vector.tensor_reduce(ssum[:], ex[:], axis=AX.X, op=ALU.add)
        rs = tmp.tile([128, 1], F32, tag="rs")
        nc.vector.reciprocal(rs[:], ssum[:])
        probs = tmp.tile([128, E], F32, tag="probs")
        nc.vector.tensor_scalar_mul(probs[:], ex[:], rs[:])
        m8 = tmp.tile([128, 8], F32, tag="m8")
        nc.vector.max(m8[:], probs[:])
        msk = tmp.tile([128, E], F32, tag="msk")
        nc.vector.tensor_scalar(msk[:], probs[:], m8[:, 1:2], None, op0=ALU.is_ge)
        nc.vector.tensor_mul(comb[:, tb, :], probs[:], msk[:])
        if not ZERO_BIASES:
            ptc = psum.tile([128, 512], F32, tag="ps")
            nc.tensor.transpose(ptc[:8, :128], comb[:, tb, :], ident[:])
            nc.vector.tensor_copy(
                comb_t[:, tb * 128:(tb + 1) * 128], ptc[:8, :128]
            )

    xpool_cm.__exit__(None, None, None)

    # ---- experts: shared first (j == -1), then routed 0..7 ----
    for j in range(-1, E):
        shared = j < 0
        # double-buffered so expert j+1's h-phase overlaps expert j's y-phase
        hbuf = hpool.tile([128, IC, T], F32R, tag="hbuf")
        if shared:
            w1d, w3d, w2d = tensors["sw1"].ap(), tensors["sw3"].ap(), tensors["sw2"].ap()
        else:
            w1d, w3d, w2d = (
                tensors["w1"].ap()[j],
                tensors["w3"].ap()[j],
                tensors["w2"].ap()[j],
            )

        s1 = []
        s3 = []
        for dc in range(DC):
            t1 = wpool.tile([128, INTER], F32R, tag="wslab")
            nc.sync.dma_start(t1[:], w1d[dc * 128:(dc + 1) * 128, :].bitcast(F32R))
            s1.append(t1)
            t3 = wpool.tile([128, INTER], F32R, tag="wslab")
            nc.sync.dma_start(t3[:], w3d[dc * 128:(dc + 1) * 128, :].bitcast(F32R))
            s3.append(t3)

        # h = silu(x @ w1 + b1) * (x @ w3 + b3), transposed layout [inter, tok]
        for nt in range(NT):
            tsl = slice(nt * 512, (nt + 1) * 512)
            for icp in range(IC // 2):
                phs = []
                for k in range(2):
                    ic = icp * 2 + k
                    icb = slice(ic * 128, (ic + 1) * 128)
                    p1 = psum.tile([128, 512], F32, tag="ps")
                    p3 = psum.tile([128, 512], F32, tag="ps")
                    for dc in range(DC):
                        st, sp = dc == 0, dc == DC - 1
                        nc.tensor.matmul(
                            p1[:], s1[dc][:, icb], xt[:, dc, tsl],
                            start=st, stop=sp,
                        )
                        nc.tensor.matmul(
                            p3[:], s3[dc][:, icb], xt[:, dc, tsl],
                            start=st, stop=sp,
                        )
                    phs.append((ic, p1, p3))
                for ic, p1, p3 in phs:
                    hs = tmp.tile([128, 512], F32, tag="hs")
                    if ZERO_BIASES:
                        if USE_SILU:
                            nc.scalar.activation(hs[:], p1[:], AF.Silu)
                        else:
                            sg = tmp.tile([128, 512], F32, tag="sg")
                            nc.scalar.activation(sg[:], p1[:], AF.Sigmoid)
                            nc.vector.tensor_mul(hs[:], sg[:], p1[:])
                        nc.vector.tensor_mul(hbuf[:, ic, tsl], hs[:], p3[:])
                        continue
                    b1c = sb1s[:, ic:ic + 1] if shared else b1s[:, j, ic:ic + 1]
                    b3c = sb3s[:, ic:ic + 1] if shared else b3s[:, j, ic:ic + 1]
                    t3v = tmp.tile([128, 512], F32, tag="t3v")
                    nc.vector.tensor_scalar_add(t3v[:], p3[:], b3c)
                    if USE_SILU:
                        nc.scalar.activation(hs[:], p1[:], AF.Silu, bias=b1c)
                    else:  # CoreSim has no Silu: silu(v) = v * sigmoid(v)
                        sg = tmp.tile([128, 512], F32, tag="sg")
                        nc.scalar.activation(sg[:], p1[:], AF.Sigmoid, bias=b1c)
                        t1v = tmp.tile([128, 512], F32, tag="t1v")
                        nc.vector.tensor_scalar_add(t1v[:], p1[:], b1c)
                        nc.vector.tensor_mul(hs[:], sg[:], t1v[:])
                    nc.vector.tensor_mul(hbuf[:, ic, tsl], hs[:], t3v[:])

        # second matmul back to natural layout + bias + weighted accumulate
        s2 = []
        for ic in range(IC):
            t2 = wpool.tile([128, D], F32R, tag="wslab")
            nc.sync.dma_start(t2[:], w2d[ic * 128:(ic + 1) * 128, :].bitcast(F32R))
            s2.append(t2)
        b2row = None if ZERO_BIASES else (sb2r[0:1, :] if shared else b2r[j:j + 1, :])
        for tb in range(TB):
            tbb = slice(tb * 128, (tb + 1) * 128)
            for dt in range(DT):
                dsl = slice(dt * 512, (dt + 1) * 512)
                py = psum.tile([128, 512], F32, tag="ps")
                for ic in range(IC):
                    nc.tensor.matmul(
                        py[:], hbuf[:, ic, tbb], s2[ic][:, dsl],
                        start=(ic == 0),
                        stop=(ic == IC - 1) and (ZERO_BIASES or not shared),
                    )
                if not ZERO_BIASES and shared:
                    # shared bias + sum_j combine[t,j]*b2[j,:] (K=8 matmul)
                    nc.tensor.matmul(
                        py[:], ones1[:], b2row[:, dsl],
                        start=False, stop=False,
                    )
                    nc.tensor.matmul(
                        py[:], comb_t[:, tbb], b2r[:, dsl],
                        start=False, stop=True,
                    )
                st = iop.tile([128, 512], F32, tag="st")
                if shared:
                    nc.scalar.copy(st[:], py[:])
                    nc.sync.dma_start(outv[:, tb, dsl], st[:])
                else:
                    # out slice += combine[:, j] * py via SWDGE accum-DMA
                    nc.vector.tensor_scalar_mul(st[:], py[:], comb[:, tb, j:j + 1])
                    nc.gpsimd.dma_start(
                        outv[:, tb, dsl], st[:], accum_op=ALU.add
                    )


def declare(nc):
    tensors = {
        "x": nc.dram_tensor("x", [T, D], F32, kind="ExternalInput"),
        "gate_w": nc.dram_tensor("gate_w", [E, D], F32, kind="ExternalInput"),
        "w1": nc.dram_tensor("w1", [E, D, INTER], F32, kind="ExternalInput"),
        "b1": nc.dram_tensor("b1", [E, INTER], F32, kind="ExternalInput"),
        "w2": nc.dram_tensor("w2", [E, INTER, D], F32, kind="ExternalInput"),
        "b2": nc.dram_tensor("b2", [E, D], F32, kind="ExternalInput"),
        "w3": nc.dram_tensor("w3", [E, D, INTER], F32, kind="ExternalInput"),
        "b3": nc.dram_tensor("b3", [E, INTER], F32, kind="ExternalInput"),
        "sw1": nc.dram_tensor("sw1", [D, INTER], F32, kind="ExternalInput"),
        "sb1": nc.dram_tensor("sb1", [INTER], F32, kind="ExternalInput"),
        "sw2": nc.dram_tensor("sw2", [INTER, D], F32, kind="ExternalInput"),
        "sb2": nc.dram_tensor("sb2", [D], F32, kind="ExternalInput"),
        "sw3": nc.dram_tensor("sw3", [D, INTER], F32, kind="ExternalInput"),
        "sb3": nc.dram_tensor("sb3", [INTER], F32, kind="ExternalInput"),
        "out": nc.dram_tensor("out", [T, D], F32, kind="ExternalOutput"),
    }
    return tensors


def build_nc(num_devices=N_CORES):
    from contextlib import ExitStack

    nc = bacc.Bacc(
        "TRN2", target_bir_lowering=False, debug=False, num_devices=num_devices
    )
    tensors = declare(nc)
    with tile.TileContext(nc) as tc:
        with ExitStack() as es:
            nc._emit_ctx = es
            emit(nc, tc, tensors)
    nc.compile()
    return nc


def make_in_maps(inputs):
    x = np.ascontiguousarray(
        np.asarray(inputs["x"], dtype=np.float32).reshape(-1, D)
    )
    shared_names = [
        "gate_w", "w1", "b1", "w2", "b2", "w3", "b3",
        "sw1", "sb1", "sw2", "sb2", "sw3", "sb3",
    ]
    shared = {
        k: np.ascontiguousarray(np.asarray(inputs[k], dtype=np.float32))
        for k in shared_names
    }
    in_maps = []
    for c in range(N_CORES):
        m = dict(shared)
        m["x"] = np.ascontiguousarray(x[c * T:(c + 1) * T])
        in_maps.append(m)
    return in_maps


def kernel(**inputs) -> np.ndarray:
    global ZERO_BIASES
    ZERO_BIASES = all(
        not np.any(np.asarray(inputs[k]))
        for k in ("b1", "b2", "b3", "sb1", "sb2", "sb3")
    )
    nc = build_nc()
    in_maps = make_in_maps(inputs)
    res = run_bass_kernel_spmd(nc, in_maps, core_ids=list(range(N_CORES)))
    out = np.concatenate([res.results[c]["out"] for c in range(N_CORES)], axis=0)
    return out.reshape(np.asarray(inputs["x"]).shape)
